# revision 22
# baseline (speedup 1.0000x reference)
"""RWKV-v4 WKV attention block on 8 Trainium2 NeuronCores.

Strategy (tensor-parallel over the channel dim, per the sharding hint):
  - Channels (dim_att = 2048) are sharded 256-per-core across 8 cores.
  - Time-shift/time-mix is folded into the projection weights on the
    host (k = x @ (tmk*Wk) + shift(x) @ ((1-tmk)*Wk)), so the device
    runs accumulating matmuls over x and its one-step-shifted view.
    (Premixing activations on the vector engines was measured to be
    3x slower than the extra matmul stream - PE is the cheap engine.)
  - Projections run in fp16 (10-bit mantissa, same precision class as
    the f32r/tf32 path, measured rmsrel ~8e-3 vs the 2e-2 budget):
    halves the x/weight DMA, halves SBUF, and runs at full PE rate.
  - WKV is fused per 512-step block, processed one block behind the
    projections so the PE never waits: PSUM-evacuating ops (exp(k),
    exp(-r) on ACT, ekv on DVE) issue at the head of the next block's
    queues, freeing the 6 PSUM banks early; the rest is SBUF-only:
        B[t] = lam_c*B[t-1] + e^{k_t} v_t   (DVE tensor_tensor_scan,
                                             carry chained across blocks)
        num[t] = B[t-1] + e^{u_c} e^{k_t} v_t
        den[t] = Bd[t-1] + e^{u_c} e^{k_t}
        den2 = den * (1 + e^{-r})           (sigmoid folded, on Pool)
        y = num * reciprocal_approx_fast(den2)
    All WKV math fp32; y is exported in fp16.
  - ONE AllToAll (fp16, 2MB) converts channel-shards to time-shards;
    each core then computes out^T = Wo^T y for its 512-step slice with
    ne-outer accumulation (1 PSUM bank), Wo streamed in fp16 chunks
    that prefetch during phase 1 on the scalar DMA queue.
"""

import sys

sys.path.insert(0, "/opt/trn_rl_repo")

import numpy as np
from contextlib import ExitStack

import concourse.bass as bass
import concourse.tile as tile
from concourse import bacc, mybir
from concourse import bass_utils

f32 = mybir.dt.float32
f16 = mybir.dt.float16
AF = mybir.ActivationFunctionType
AL = mybir.AluOpType

T, C, NE = 4096, 2048, 2048
N_CORES = 8
C_LOC = C // N_CORES          # 256 channels per core
T_SH = T // N_CORES           # 512-step output shard per core
N_NE = NE // 128              # 16 contraction chunks
TB = 512                      # projection t-block == WKV chunk == t-shard
NTB = T // TB

W_NAMES = ("w1k", "w2k", "w1v", "w2v", "w1r", "w2r")


def build_program():
    nc = bacc.Bacc("TRN2", target_bir_lowering=False, debug=False,
                   num_devices=N_CORES)

    xT = nc.dram_tensor("xT", [NE, T + 1], f16, kind="ExternalInput")
    w_dram = {n: nc.dram_tensor(n, [NE, C_LOC], f16, kind="ExternalInput")
              for n in W_NAMES}
    wo4 = nc.dram_tensor("wo4", [NE, C], f16, kind="ExternalInput")
    lamb = nc.dram_tensor("lamb", [C_LOC, TB], f32, kind="ExternalInput")
    eu_in = nc.dram_tensor("eu", [C_LOC, 1], f32, kind="ExternalInput")
    outT = nc.dram_tensor("out", [NE, T_SH], f32, kind="ExternalOutput")

    a2a_in = [nc.dram_tensor(f"a2a_in{g}", [C // 2, T_SH], f16,
                             kind="Internal") for g in range(2)]
    a2a_out = [nc.dram_tensor(f"a2a_out{g}", [C // 2, T_SH], f16,
                              kind="Internal") for g in range(2)]

    xT_r = xT.ap().rearrange("(n p) t -> p n t", p=128)

    with tile.TileContext(nc) as tc:
        with ExitStack() as octx:
            cpool = octx.enter_context(tc.tile_pool(name="const", bufs=1))
            wopool = octx.enter_context(tc.tile_pool(name="wo", bufs=8))
            yshpool = octx.enter_context(tc.tile_pool(name="ysh", bufs=1))
            obounce = octx.enter_context(tc.tile_pool(name="ob", bufs=2))

            # ---- constants (scalar DMA queue - tiny) ----------------------
            lam_t, eu_t = {}, {}
            for g in range(2):
                cs = slice(g * 128, (g + 1) * 128)
                lt = cpool.tile([128, TB], f32, tag=f"lam{g}", name=f"lam{g}")
                nc.scalar.dma_start(lt[:], lamb.ap()[cs, :])
                lam_t[g] = lt
                et = cpool.tile([128, 1], f32, tag=f"eu{g}", name=f"eu{g}")
                nc.scalar.dma_start(et[:], eu_in.ap()[cs, :])
                eu_t[g] = et

            # ---- phase-2 Wo stream: 4 chunks prefetch during phase 1 ------
            wo_r = wo4.ap().rearrange("(ne p) (c j) -> ne p c j",
                                      ne=N_NE, c=N_NE)
            wo_sb = {}

            def load_wo(ne):
                wt = wopool.tile([128, N_NE, 128], f16, tag="wo",
                                 name=f"wo{ne}")
                nc.scalar.dma_start(wt[:], wo_r[ne])
                wo_sb[ne] = wt

            for ne in range(8):
                load_wo(ne)

            with ExitStack() as p1:
                psA = p1.enter_context(
                    tc.tile_pool(name="psA", bufs=8, space="PSUM"))
                wpool = p1.enter_context(tc.tile_pool(name="wpool", bufs=1))
                slab_pool = p1.enter_context(
                    tc.tile_pool(name="slab", bufs=20))
                wkp = p1.enter_context(tc.tile_pool(name="wkv", bufs=2))
                scp = p1.enter_context(tc.tile_pool(name="scan", bufs=2))

                # projection weights (fp16), quarter-chunked and issued
                # q0-of-all-six first so the first matmuls start early
                w_sb = {n: wpool.tile([128, N_NE, C_LOC], f16, tag=n, name=n)
                        for n in W_NAMES}
                w_src = {n: w_dram[n].ap().rearrange("(n p) c -> p n c",
                                                     p=128)
                         for n in W_NAMES}
                for q in range(4):
                    for n in W_NAMES:
                        nc.gpsimd.dma_start(
                            w_sb[n][:, q * 4:(q + 1) * 4, :],
                            w_src[n][:, q * 4:(q + 1) * 4, :])

                def issue_slabs(tb):
                    t0 = tb * TB
                    slabs = []
                    for ne in range(N_NE):
                        s = slab_pool.tile([128, TB + 1], f16, tag="slab",
                                           name=f"slab{tb}_{ne}")
                        nc.sync.dma_start(s[:], xT_r[:, ne, t0:t0 + TB + 1])
                        slabs.append(s)
                    return slabs

                slabs_next = issue_slabs(0)

                carry = {}   # (g, 'n'/'d') -> [128,1] sbuf carry tiles
                for g in range(2):
                    for which in "nd":
                        ct = cpool.tile([128, 1], f32, tag=f"c{which}{g}",
                                        name=f"c{which}{g}")
                        carry[(g, which)] = ct

                projs = (("w1k", "w2k"), ("w1v", "w2v"), ("w1r", "w2r"))

                def wkv_evac(tb, pp):
                    """PSUM-evacuating ops for block tb: head of the next
                    block's ACT/DVE queues frees the 6 banks early."""
                    ev = {}
                    for g in range(2):
                        ek = wkp.tile([128, TB], f32, tag="ek",
                                      name=f"ek{tb}_{g}")
                        nc.scalar.activation(ek[:], pp[(0, g)][:], AF.Exp)
                        emr = wkp.tile([128, TB], f32, tag="emr",
                                       name=f"emr{tb}_{g}")
                        nc.scalar.activation(emr[:], pp[(2, g)][:], AF.Exp,
                                             scale=-1.0)
                        ekv = wkp.tile([128, TB], f32, tag="ekv",
                                       name=f"ekv{tb}_{g}")
                        nc.vector.tensor_mul(ekv[:], ek[:], pp[(1, g)][:])
                        ev[g] = (ek, emr, ekv)
                    return ev

                def wkv_rest(tb, ev):
                    """Post-PSUM WKV math for block tb (SBUF-only)."""
                    for g in range(2):
                        ek, emr, ekv = ev[g]
                        bn = scp.tile([128, TB], f32, tag="bn",
                                      name=f"bn{tb}_{g}")
                        bd = scp.tile([128, TB], f32, tag="bd",
                                      name=f"bd{tb}_{g}")
                        init_n = 0.0 if tb == 0 else carry[(g, "n")][:]
                        init_d = 0.0 if tb == 0 else carry[(g, "d")][:]
                        nc.vector.tensor_tensor_scan(
                            bn[:], lam_t[g][:], ekv[:], init_n,
                            AL.mult, AL.add)
                        nc.vector.tensor_tensor_scan(
                            bd[:], lam_t[g][:], ek[:], init_d,
                            AL.mult, AL.add)

                        num = wkp.tile([128, TB], f32, tag="num",
                                       name=f"num{tb}_{g}")
                        den = wkp.tile([128, TB], f32, tag="den",
                                       name=f"den{tb}_{g}")
                        nc.vector.scalar_tensor_tensor(
                            num[:, 1:TB], ekv[:, 1:TB], eu_t[g][:],
                            bn[:, 0:TB - 1], AL.mult, AL.add)
                        nc.vector.scalar_tensor_tensor(
                            den[:, 1:TB], ek[:, 1:TB], eu_t[g][:],
                            bd[:, 0:TB - 1], AL.mult, AL.add)
                        if tb == 0:
                            nc.vector.tensor_scalar_mul(
                                num[:, 0:1], ekv[:, 0:1], eu_t[g][:])
                            nc.vector.tensor_scalar_mul(
                                den[:, 0:1], ek[:, 0:1], eu_t[g][:])
                        else:
                            nc.vector.scalar_tensor_tensor(
                                num[:, 0:1], ekv[:, 0:1], eu_t[g][:],
                                carry[(g, "n")][:], AL.mult, AL.add)
                            nc.vector.scalar_tensor_tensor(
                                den[:, 0:1], ek[:, 0:1], eu_t[g][:],
                                carry[(g, "d")][:], AL.mult, AL.add)
                        if tb < NTB - 1:
                            nc.vector.tensor_copy(carry[(g, "n")][:],
                                                  bn[:, TB - 1:TB])
                            nc.vector.tensor_copy(carry[(g, "d")][:],
                                                  bd[:, TB - 1:TB])

                        # den2 = den * (1 + e^{-r}) on Pool (sigmoid folded
                        # into the denominator); emr's buffer takes the
                        # product, ekv's (dead by now) takes den2, then one
                        # fast reciprocal (DVE-only custom op) into emr
                        nc.gpsimd.tensor_mul(emr[:], emr[:], den[:])
                        nc.gpsimd.tensor_add(ekv[:], emr[:], den[:])
                        nc.vector.reciprocal_approx_fast(emr[:], ekv[:])
                        y_t = wkp.tile([128, TB], f16, tag="y",
                                       name=f"y{tb}_{g}")
                        nc.vector.tensor_mul(y_t[:], num[:], emr[:])

                        r0 = tb * 128
                        nc.sync.dma_start(a2a_in[g].ap()[r0:r0 + 128, :],
                                          y_t[:])

                pp_prev = None
                for tb in range(NTB):
                    slabs = slabs_next
                    if tb < NTB - 1:
                        slabs_next = issue_slabs(tb + 1)

                    ev_prev = (wkv_evac(tb - 1, pp_prev)
                               if pp_prev is not None else None)

                    pp = {}
                    for pi in range(3):
                        for g in range(2):
                            pp[(pi, g)] = psA.tile(
                                [128, TB], f32, tag="proj",
                                name=f"proj{tb}_{pi}_{g}")

                    for ne in range(N_NE):
                        s = slabs[ne]
                        for pi, (n1, n2) in enumerate(projs):
                            for g in range(2):
                                gsl = slice(g * 128, (g + 1) * 128)
                                nc.tensor.matmul(
                                    pp[(pi, g)][:], w_sb[n1][:, ne, gsl],
                                    s[:, 1:TB + 1],
                                    start=(ne == 0), stop=False)
                                nc.tensor.matmul(
                                    pp[(pi, g)][:], w_sb[n2][:, ne, gsl],
                                    s[:, 0:TB],
                                    start=False, stop=(ne == N_NE - 1))

                    if ev_prev is not None:
                        wkv_rest(tb - 1, ev_prev)
                    pp_prev = pp

                wkv_rest(NTB - 1, wkv_evac(NTB - 1, pp_prev))

            # ---------------- AllToAll + output projection ------------------
            # two collectives (one per channel group): the second pipelines
            # behind the first, and phase 2 starts on g0 channels while the
            # g1 collective is still in flight
            for g in range(2):
                nc.gpsimd.collective_compute(
                    "AllToAll", AL.bypass,
                    replica_groups=[list(range(N_CORES))],
                    ins=[a2a_in[g].ap().opt()], outs=[a2a_out[g].ap().opt()],
                )

            with ExitStack() as p3:
                psB = p3.enter_context(
                    tc.tile_pool(name="psB", bufs=8, space="PSUM"))
                ysh_g = {}
                for g in range(2):
                    yt = yshpool.tile([128, 8, T_SH], f16, tag=f"ysh{g}",
                                      name=f"ysh{g}")
                    ao_r = a2a_out[g].ap().rearrange("(c p) t -> p c t",
                                                     p=128)
                    eng = nc.gpsimd if g == 0 else nc.sync
                    for csrc in range(8):
                        eng.dma_start(yt[:, csrc, :], ao_r[:, csrc, :])
                    ysh_g[g] = yt

                def ysh_ap(c):
                    return ysh_g[c % 2][:, c // 2, :]

                evens = list(range(0, N_NE, 2))
                odds = list(range(1, N_NE, 2))
                psb = {}
                for ne in range(8):
                    psb[ne] = psB.tile([128, T_SH], f32, tag="fin",
                                       name=f"fin{ne}")
                    for c in evens:
                        nc.tensor.matmul(
                            psb[ne][:], wo_sb[ne][:, c, :], ysh_ap(c),
                            start=(c == 0), stop=False)

                def finish(ne, corder):
                    ps = psb[ne]
                    for c in corder:
                        nc.tensor.matmul(
                            ps[:], wo_sb[ne][:, c, :], ysh_ap(c),
                            start=False, stop=(c == corder[-1]))
                    b = obounce.tile([128, T_SH], f32, tag="evict",
                                     name=f"ob{ne}")
                    nc.scalar.copy(b[:], ps[:])
                    nc.sync.dma_start(
                        outT.ap()[ne * 128:(ne + 1) * 128, :], b[:])

                for ne in range(8):
                    finish(ne, odds)
                    load_wo(ne + 8)
                for ne in range(8, N_NE):
                    psb[ne] = psB.tile([128, T_SH], f32, tag="fin",
                                       name=f"fin{ne}")
                    for c in evens:
                        nc.tensor.matmul(
                            psb[ne][:], wo_sb[ne][:, c, :], ysh_ap(c),
                            start=(c == 0), stop=False)
                    finish(ne, odds)

    nc.compile()
    return nc


_prog_cache = {}


def _get_program():
    if "nc" not in _prog_cache:
        _prog_cache["nc"] = build_program()
    return _prog_cache["nc"]


def prepare_in_maps(x, time_first, time_decay, time_mix_k, time_mix_v,
                    time_mix_r, Wk, Wv, Wr, Wo):
    x = np.asarray(x, np.float32)
    tmk = np.asarray(time_mix_k, np.float32).reshape(-1)
    tmv = np.asarray(time_mix_v, np.float32).reshape(-1)
    tmr = np.asarray(time_mix_r, np.float32).reshape(-1)
    td = np.asarray(time_decay, np.float64).reshape(-1)
    tf_ = np.asarray(time_first, np.float64).reshape(-1)

    xT = np.zeros((NE, T + 1), np.float16)            # col 0 = shift-in zero
    xT[:, 1:] = x.T.astype(np.float16)
    Wk = np.asarray(Wk, np.float32)
    Wv = np.asarray(Wv, np.float32)
    Wr = np.asarray(Wr, np.float32)
    folded = {
        "w1k": tmk[:, None] * Wk, "w2k": (1.0 - tmk)[:, None] * Wk,
        "w1v": tmv[:, None] * Wv, "w2v": (1.0 - tmv)[:, None] * Wv,
        "w1r": tmr[:, None] * Wr, "w2r": (1.0 - tmr)[:, None] * Wr,
    }
    # wo4[ne*128+p, nc*128+j] = Wo[nc*128+p, ne*128+j]
    wo4 = np.ascontiguousarray(
        np.asarray(Wo, np.float32).astype(np.float16)
        .reshape(N_NE, 128, N_NE, 128).transpose(2, 1, 0, 3)
        .reshape(NE, C))
    # per-step decay multiplier lam_c = exp(-exp(-exp(time_decay_c)))
    lam = np.exp(-np.exp(-np.exp(td))).astype(np.float32)
    eu = np.exp(tf_).astype(np.float32)

    in_maps = []
    for i in range(N_CORES):
        sl = slice(i * C_LOC, (i + 1) * C_LOC)
        im = {
            "xT": xT,
            "wo4": wo4,
            "lamb": np.ascontiguousarray(
                np.broadcast_to(lam[sl, None], (C_LOC, TB))),
            "eu": np.ascontiguousarray(eu[sl, None]),
        }
        for n, W in folded.items():
            im[n] = np.ascontiguousarray(W[:, sl].astype(np.float16))
        in_maps.append(im)
    return in_maps


def run(in_maps, trace=False, **kwargs):
    nc = _get_program()
    res = bass_utils.run_bass_kernel_spmd(
        nc, in_maps, core_ids=list(range(N_CORES)), trace=trace, **kwargs)
    full = np.concatenate(
        [np.asarray(res.results[i]["out"], np.float32).T
         for i in range(N_CORES)], axis=0)
    return full, res


def kernel(**inputs):
    in_maps = prepare_in_maps(**inputs)
    full, _ = run(in_maps, trace=False)
    return full


# revision 23
# speedup vs baseline: 1.0227x; 1.0227x over previous
"""RWKV-v4 WKV attention block on 8 Trainium2 NeuronCores.

Strategy (tensor-parallel over the channel dim, per the sharding hint):
  - Channels (dim_att = 2048) are sharded 256-per-core across 8 cores.
  - Time-shift/time-mix is folded into the projection weights on the
    host (k = x @ (tmk*Wk) + shift(x) @ ((1-tmk)*Wk)), so the device
    runs accumulating matmuls over x and its one-step-shifted view.
    (Premixing activations on the vector engines was measured to be
    3x slower than the extra matmul stream - PE is the cheap engine.)
  - Projections run in fp16 (10-bit mantissa, same precision class as
    the f32r/tf32 path, measured rmsrel ~8e-3 vs the 2e-2 budget):
    halves the x/weight DMA, halves SBUF, and runs at full PE rate.
  - WKV is fused per 512-step block, processed one block behind the
    projections so the PE never waits: PSUM-evacuating ops (exp(k),
    exp(-r) on ACT, ekv on DVE) issue at the head of the next block's
    queues, freeing the 6 PSUM banks early; the rest is SBUF-only:
        B[t] = lam_c*B[t-1] + e^{k_t} v_t   (DVE tensor_tensor_scan,
                                             carry chained across blocks)
        num[t] = B[t-1] + e^{u_c} e^{k_t} v_t
        den[t] = Bd[t-1] + e^{u_c} e^{k_t}
        den2 = den * (1 + e^{-r})           (sigmoid folded, on Pool)
        y = num * reciprocal_approx_fast(den2)
    All WKV math fp32; y is exported in fp16.
  - ONE AllToAll (fp16, 2MB) converts channel-shards to time-shards;
    each core then computes out^T = Wo^T y for its 512-step slice with
    ne-outer accumulation (1 PSUM bank), Wo streamed in fp16 chunks
    that prefetch during phase 1 on the scalar DMA queue.
"""

import sys

sys.path.insert(0, "/opt/trn_rl_repo")

import numpy as np
from contextlib import ExitStack

import concourse.bass as bass
import concourse.tile as tile
from concourse import bacc, mybir
from concourse import bass_utils

f32 = mybir.dt.float32
f16 = mybir.dt.float16
AF = mybir.ActivationFunctionType
AL = mybir.AluOpType

T, C, NE = 4096, 2048, 2048
N_CORES = 8
C_LOC = C // N_CORES          # 256 channels per core
T_SH = T // N_CORES           # 512-step output shard per core
N_NE = NE // 128              # 16 contraction chunks
TB = 512                      # projection t-block == WKV chunk == t-shard
NTB = T // TB

W_NAMES = ("w1k", "w2k", "w1v", "w2v", "w1r", "w2r")


def build_program():
    nc = bacc.Bacc("TRN2", target_bir_lowering=False, debug=False,
                   num_devices=N_CORES)

    xT = nc.dram_tensor("xT", [NE, T + 1], f16, kind="ExternalInput")
    w_dram = {n: nc.dram_tensor(n, [NE, C_LOC], f16, kind="ExternalInput")
              for n in W_NAMES}
    wo4 = nc.dram_tensor("wo4", [NE, C], f16, kind="ExternalInput")
    lamb = nc.dram_tensor("lamb", [C_LOC, TB], f32, kind="ExternalInput")
    eu_in = nc.dram_tensor("eu", [C_LOC, 1], f32, kind="ExternalInput")
    outT = nc.dram_tensor("out", [NE, T_SH], f32, kind="ExternalOutput")

    a2a_in = [nc.dram_tensor(f"a2a_in{g}", [C // 2, T_SH], f16,
                             kind="Internal") for g in range(2)]
    a2a_out = [nc.dram_tensor(f"a2a_out{g}", [C // 2, T_SH], f16,
                              kind="Internal") for g in range(2)]

    xT_r = xT.ap().rearrange("(n p) t -> p n t", p=128)

    with tile.TileContext(nc) as tc:
        with ExitStack() as octx:
            cpool = octx.enter_context(tc.tile_pool(name="const", bufs=1))
            wopool = octx.enter_context(tc.tile_pool(name="wo", bufs=8))
            yshpool = octx.enter_context(tc.tile_pool(name="ysh", bufs=1))
            obounce = octx.enter_context(tc.tile_pool(name="ob", bufs=2))

            # ---- constants (scalar DMA queue - tiny) ----------------------
            lam_t, eu_t = {}, {}
            for g in range(2):
                cs = slice(g * 128, (g + 1) * 128)
                lt = cpool.tile([128, TB], f32, tag=f"lam{g}", name=f"lam{g}")
                nc.scalar.dma_start(lt[:], lamb.ap()[cs, :])
                lam_t[g] = lt
                et = cpool.tile([128, 1], f32, tag=f"eu{g}", name=f"eu{g}")
                nc.scalar.dma_start(et[:], eu_in.ap()[cs, :])
                eu_t[g] = et

            # ---- phase-2 Wo stream: 4 chunks prefetch during phase 1 ------
            wo_r = wo4.ap().rearrange("(ne p) (c j) -> ne p c j",
                                      ne=N_NE, c=N_NE)
            wo_sb = {}

            def load_wo(ne):
                wt = wopool.tile([128, N_NE, 128], f16, tag="wo",
                                 name=f"wo{ne}")
                nc.scalar.dma_start(wt[:], wo_r[ne])
                wo_sb[ne] = wt

            for ne in range(8):
                load_wo(ne)

            with ExitStack() as p1:
                psA = p1.enter_context(
                    tc.tile_pool(name="psA", bufs=8, space="PSUM"))
                wpool = p1.enter_context(tc.tile_pool(name="wpool", bufs=1))
                slab_pool = p1.enter_context(
                    tc.tile_pool(name="slab", bufs=20))
                wkp = p1.enter_context(tc.tile_pool(name="wkv", bufs=2))
                scp = p1.enter_context(tc.tile_pool(name="scan", bufs=2))

                # projection weights (fp16), quarter-chunked and issued
                # q0-of-all-six first so the first matmuls start early
                w_sb = {n: wpool.tile([128, N_NE, C_LOC], f16, tag=n, name=n)
                        for n in W_NAMES}
                w_src = {n: w_dram[n].ap().rearrange("(n p) c -> p n c",
                                                     p=128)
                         for n in W_NAMES}
                for q in range(4):
                    for n in W_NAMES:
                        nc.gpsimd.dma_start(
                            w_sb[n][:, q * 4:(q + 1) * 4, :],
                            w_src[n][:, q * 4:(q + 1) * 4, :])

                def issue_slabs(tb):
                    t0 = tb * TB
                    slabs = []
                    for ne in range(N_NE):
                        s = slab_pool.tile([128, TB + 1], f16, tag="slab",
                                           name=f"slab{tb}_{ne}")
                        nc.sync.dma_start(s[:], xT_r[:, ne, t0:t0 + TB + 1])
                        slabs.append(s)
                    return slabs

                slabs_next = issue_slabs(0)

                carry = {}   # (g, 'n'/'d') -> [128,1] sbuf carry tiles
                for g in range(2):
                    for which in "nd":
                        ct = cpool.tile([128, 1], f32, tag=f"c{which}{g}",
                                        name=f"c{which}{g}")
                        carry[(g, which)] = ct

                projs = (("w1k", "w2k"), ("w1v", "w2v"), ("w1r", "w2r"))

                def wkv_evac(tb, pp):
                    """PSUM-evacuating ops for block tb: head of the next
                    block's ACT/DVE queues frees the 6 banks early."""
                    ev = {}
                    for g in range(2):
                        ek = wkp.tile([128, TB], f32, tag="ek",
                                      name=f"ek{tb}_{g}")
                        nc.scalar.activation(ek[:], pp[(0, g)][:], AF.Exp)
                        emr = wkp.tile([128, TB], f32, tag="emr",
                                       name=f"emr{tb}_{g}")
                        nc.scalar.activation(emr[:], pp[(2, g)][:], AF.Exp,
                                             scale=-1.0)
                        ekv = wkp.tile([128, TB], f32, tag="ekv",
                                       name=f"ekv{tb}_{g}")
                        nc.vector.tensor_mul(ekv[:], ek[:], pp[(1, g)][:])
                        ev[g] = (ek, emr, ekv)
                    return ev

                def wkv_rest(tb, ev):
                    """Post-PSUM WKV math for block tb (SBUF-only)."""
                    for g in range(2):
                        ek, emr, ekv = ev[g]
                        bn = scp.tile([128, TB], f32, tag="bn",
                                      name=f"bn{tb}_{g}")
                        bd = scp.tile([128, TB], f32, tag="bd",
                                      name=f"bd{tb}_{g}")
                        init_n = 0.0 if tb == 0 else carry[(g, "n")][:]
                        init_d = 0.0 if tb == 0 else carry[(g, "d")][:]
                        nc.vector.tensor_tensor_scan(
                            bn[:], lam_t[g][:], ekv[:], init_n,
                            AL.mult, AL.add)
                        nc.vector.tensor_tensor_scan(
                            bd[:], lam_t[g][:], ek[:], init_d,
                            AL.mult, AL.add)

                        num = wkp.tile([128, TB], f32, tag="num",
                                       name=f"num{tb}_{g}")
                        den = wkp.tile([128, TB], f32, tag="den",
                                       name=f"den{tb}_{g}")
                        nc.vector.scalar_tensor_tensor(
                            num[:, 1:TB], ekv[:, 1:TB], eu_t[g][:],
                            bn[:, 0:TB - 1], AL.mult, AL.add)
                        nc.vector.scalar_tensor_tensor(
                            den[:, 1:TB], ek[:, 1:TB], eu_t[g][:],
                            bd[:, 0:TB - 1], AL.mult, AL.add)
                        if tb == 0:
                            nc.vector.tensor_scalar_mul(
                                num[:, 0:1], ekv[:, 0:1], eu_t[g][:])
                            nc.vector.tensor_scalar_mul(
                                den[:, 0:1], ek[:, 0:1], eu_t[g][:])
                        else:
                            nc.vector.scalar_tensor_tensor(
                                num[:, 0:1], ekv[:, 0:1], eu_t[g][:],
                                carry[(g, "n")][:], AL.mult, AL.add)
                            nc.vector.scalar_tensor_tensor(
                                den[:, 0:1], ek[:, 0:1], eu_t[g][:],
                                carry[(g, "d")][:], AL.mult, AL.add)
                        if tb < NTB - 1:
                            nc.vector.tensor_copy(carry[(g, "n")][:],
                                                  bn[:, TB - 1:TB])
                            nc.vector.tensor_copy(carry[(g, "d")][:],
                                                  bd[:, TB - 1:TB])

                        # den2 = den * (1 + e^{-r}) on Pool (sigmoid folded
                        # into the denominator); emr's buffer takes the
                        # product, ekv's (dead by now) takes den2, then one
                        # fast reciprocal (DVE-only custom op) into emr
                        nc.gpsimd.tensor_mul(emr[:], emr[:], den[:])
                        nc.gpsimd.tensor_add(ekv[:], emr[:], den[:])
                        nc.vector.reciprocal_approx_fast(emr[:], ekv[:])
                        y_t = wkp.tile([128, TB], f16, tag="y",
                                       name=f"y{tb}_{g}")
                        nc.vector.tensor_mul(y_t[:], num[:], emr[:])

                        r0 = tb * 128
                        nc.sync.dma_start(a2a_in[g].ap()[r0:r0 + 128, :],
                                          y_t[:])

                pp_prev = None
                for tb in range(NTB):
                    slabs = slabs_next
                    if tb < NTB - 1:
                        slabs_next = issue_slabs(tb + 1)

                    ev_prev = (wkv_evac(tb - 1, pp_prev)
                               if pp_prev is not None else None)

                    pp = {}
                    for pi in range(3):
                        for g in range(2):
                            pp[(pi, g)] = psA.tile(
                                [128, TB], f32, tag="proj",
                                name=f"proj{tb}_{pi}_{g}")

                    for ne in range(N_NE):
                        s = slabs[ne]
                        for pi, (n1, n2) in enumerate(projs):
                            for g in range(2):
                                gsl = slice(g * 128, (g + 1) * 128)
                                nc.tensor.matmul(
                                    pp[(pi, g)][:], w_sb[n1][:, ne, gsl],
                                    s[:, 1:TB + 1],
                                    start=(ne == 0), stop=False)
                                nc.tensor.matmul(
                                    pp[(pi, g)][:], w_sb[n2][:, ne, gsl],
                                    s[:, 0:TB],
                                    start=False, stop=(ne == N_NE - 1))

                    if ev_prev is not None:
                        wkv_rest(tb - 1, ev_prev)
                    pp_prev = pp

                wkv_rest(NTB - 1, wkv_evac(NTB - 1, pp_prev))

            # ---------------- AllToAll + output projection ------------------
            # two collectives (one per channel group): the second pipelines
            # behind the first, and phase 2 starts on g0 channels while the
            # g1 collective is still in flight
            for g in range(2):
                nc.gpsimd.collective_compute(
                    "AllToAll", AL.bypass,
                    replica_groups=[list(range(N_CORES))],
                    ins=[a2a_in[g].ap().opt()], outs=[a2a_out[g].ap().opt()],
                )

            with ExitStack() as p3:
                psB = p3.enter_context(
                    tc.tile_pool(name="psB", bufs=8, space="PSUM"))
                ysh_g = {}
                for g in range(2):
                    yt = yshpool.tile([128, 8, T_SH], f16, tag=f"ysh{g}",
                                      name=f"ysh{g}")
                    ao_r = a2a_out[g].ap().rearrange("(c p) t -> p c t",
                                                     p=128)
                    for csrc in range(8):
                        nc.sync.dma_start(yt[:, csrc, :], ao_r[:, csrc, :])
                    ysh_g[g] = yt

                def ysh_ap(c):
                    return ysh_g[c % 2][:, c // 2, :]

                evens = list(range(0, N_NE, 2))
                odds = list(range(1, N_NE, 2))
                psb = {}
                for ne in range(8):
                    psb[ne] = psB.tile([128, T_SH], f32, tag="fin",
                                       name=f"fin{ne}")
                    for c in evens:
                        nc.tensor.matmul(
                            psb[ne][:], wo_sb[ne][:, c, :], ysh_ap(c),
                            start=(c == 0), stop=False)

                def finish(ne, corder):
                    ps = psb[ne]
                    for c in corder:
                        nc.tensor.matmul(
                            ps[:], wo_sb[ne][:, c, :], ysh_ap(c),
                            start=False, stop=(c == corder[-1]))
                    b = obounce.tile([128, T_SH], f32, tag="evict",
                                     name=f"ob{ne}")
                    nc.scalar.copy(b[:], ps[:])
                    nc.sync.dma_start(
                        outT.ap()[ne * 128:(ne + 1) * 128, :], b[:])

                for ne in range(8):
                    finish(ne, odds)
                    load_wo(ne + 8)
                for ne in range(8, N_NE):
                    psb[ne] = psB.tile([128, T_SH], f32, tag="fin",
                                       name=f"fin{ne}")
                    for c in evens:
                        nc.tensor.matmul(
                            psb[ne][:], wo_sb[ne][:, c, :], ysh_ap(c),
                            start=(c == 0), stop=False)
                    finish(ne, odds)

    nc.compile()
    return nc


_prog_cache = {}


def _get_program():
    if "nc" not in _prog_cache:
        _prog_cache["nc"] = build_program()
    return _prog_cache["nc"]


def prepare_in_maps(x, time_first, time_decay, time_mix_k, time_mix_v,
                    time_mix_r, Wk, Wv, Wr, Wo):
    x = np.asarray(x, np.float32)
    tmk = np.asarray(time_mix_k, np.float32).reshape(-1)
    tmv = np.asarray(time_mix_v, np.float32).reshape(-1)
    tmr = np.asarray(time_mix_r, np.float32).reshape(-1)
    td = np.asarray(time_decay, np.float64).reshape(-1)
    tf_ = np.asarray(time_first, np.float64).reshape(-1)

    xT = np.zeros((NE, T + 1), np.float16)            # col 0 = shift-in zero
    xT[:, 1:] = x.T.astype(np.float16)
    Wk = np.asarray(Wk, np.float32)
    Wv = np.asarray(Wv, np.float32)
    Wr = np.asarray(Wr, np.float32)
    folded = {
        "w1k": tmk[:, None] * Wk, "w2k": (1.0 - tmk)[:, None] * Wk,
        "w1v": tmv[:, None] * Wv, "w2v": (1.0 - tmv)[:, None] * Wv,
        "w1r": tmr[:, None] * Wr, "w2r": (1.0 - tmr)[:, None] * Wr,
    }
    # wo4[ne*128+p, nc*128+j] = Wo[nc*128+p, ne*128+j]
    wo4 = np.ascontiguousarray(
        np.asarray(Wo, np.float32).astype(np.float16)
        .reshape(N_NE, 128, N_NE, 128).transpose(2, 1, 0, 3)
        .reshape(NE, C))
    # per-step decay multiplier lam_c = exp(-exp(-exp(time_decay_c)))
    lam = np.exp(-np.exp(-np.exp(td))).astype(np.float32)
    eu = np.exp(tf_).astype(np.float32)

    in_maps = []
    for i in range(N_CORES):
        sl = slice(i * C_LOC, (i + 1) * C_LOC)
        im = {
            "xT": xT,
            "wo4": wo4,
            "lamb": np.ascontiguousarray(
                np.broadcast_to(lam[sl, None], (C_LOC, TB))),
            "eu": np.ascontiguousarray(eu[sl, None]),
        }
        for n, W in folded.items():
            im[n] = np.ascontiguousarray(W[:, sl].astype(np.float16))
        in_maps.append(im)
    return in_maps


def run(in_maps, trace=False, **kwargs):
    nc = _get_program()
    res = bass_utils.run_bass_kernel_spmd(
        nc, in_maps, core_ids=list(range(N_CORES)), trace=trace, **kwargs)
    full = np.concatenate(
        [np.asarray(res.results[i]["out"], np.float32).T
         for i in range(N_CORES)], axis=0)
    return full, res


def kernel(**inputs):
    in_maps = prepare_in_maps(**inputs)
    full, _ = run(in_maps, trace=False)
    return full


# revision 24
# speedup vs baseline: 1.0626x; 1.0390x over previous
"""RWKV-v4 WKV attention block on 8 Trainium2 NeuronCores.

Strategy (tensor-parallel over the channel dim, per the sharding hint):
  - Channels (dim_att = 2048) are sharded 256-per-core across 8 cores.
  - Time-shift/time-mix is folded into the projection weights on the
    host (k = x @ (tmk*Wk) + shift(x) @ ((1-tmk)*Wk)), so the device
    runs accumulating matmuls over x and its one-step-shifted view.
    (Premixing activations on the vector engines was measured to be
    3x slower than the extra matmul stream - PE is the cheap engine.)
  - Projections run in fp16 (10-bit mantissa, same precision class as
    the f32r/tf32 path, measured rmsrel ~8e-3 vs the 2e-2 budget):
    halves the x/weight DMA, halves SBUF, and runs at full PE rate.
  - WKV is fused per 512-step block, processed one block behind the
    projections so the PE never waits: PSUM-evacuating ops (exp(k),
    exp(-r) on ACT, ekv on DVE) issue at the head of the next block's
    queues, freeing the 6 PSUM banks early; the rest is SBUF-only:
        B[t] = lam_c*B[t-1] + e^{k_t} v_t   (DVE tensor_tensor_scan,
                                             carry chained across blocks)
        num[t] = B[t-1] + e^{u_c} e^{k_t} v_t
        den[t] = Bd[t-1] + e^{u_c} e^{k_t}
        den2 = den * (1 + e^{-r})           (sigmoid folded, on Pool)
        y = num * reciprocal_approx_fast(den2)
    All WKV math fp32; y is exported in fp16.
  - ONE AllToAll (fp16, 2MB) converts channel-shards to time-shards;
    each core then computes out^T = Wo^T y for its 512-step slice with
    ne-outer accumulation (1 PSUM bank), Wo streamed in fp16 chunks
    that prefetch during phase 1 on the scalar DMA queue.
"""

import sys

sys.path.insert(0, "/opt/trn_rl_repo")

import numpy as np
from contextlib import ExitStack

import concourse.bass as bass
import concourse.tile as tile
from concourse import bacc, mybir
from concourse import bass_utils

f32 = mybir.dt.float32
f16 = mybir.dt.float16
AF = mybir.ActivationFunctionType
AL = mybir.AluOpType

T, C, NE = 4096, 2048, 2048
N_CORES = 8
C_LOC = C // N_CORES          # 256 channels per core
T_SH = T // N_CORES           # 512-step output shard per core
N_NE = NE // 128              # 16 contraction chunks
TB = 512                      # projection t-block == WKV chunk == t-shard
NTB = T // TB

W_NAMES = ("w1k", "w2k", "w1v", "w2v", "w1r", "w2r")


def build_program():
    nc = bacc.Bacc("TRN2", target_bir_lowering=False, debug=False,
                   num_devices=N_CORES)

    xT = nc.dram_tensor("xT", [NE, T + 1], f16, kind="ExternalInput")
    w_dram = {n: nc.dram_tensor(n, [NE, C_LOC], f16, kind="ExternalInput")
              for n in W_NAMES}
    wo4 = nc.dram_tensor("wo4", [NE, C], f16, kind="ExternalInput")
    lamb = nc.dram_tensor("lamb", [C_LOC, TB], f32, kind="ExternalInput")
    eu_in = nc.dram_tensor("eu", [C_LOC, 1], f32, kind="ExternalInput")
    outT = nc.dram_tensor("out", [NE, T_SH], f32, kind="ExternalOutput")

    a2a_in = [nc.dram_tensor(f"a2a_in{g}", [C // 2, T_SH], f16,
                             kind="Internal") for g in range(2)]
    a2a_out = [nc.dram_tensor(f"a2a_out{g}", [C // 2, T_SH], f16,
                              kind="Internal") for g in range(2)]

    xT_r = xT.ap().rearrange("(n p) t -> p n t", p=128)

    with tile.TileContext(nc) as tc:
        with ExitStack() as octx:
            cpool = octx.enter_context(tc.tile_pool(name="const", bufs=1))
            wopool = octx.enter_context(tc.tile_pool(name="wo", bufs=8))
            yshpool = octx.enter_context(tc.tile_pool(name="ysh", bufs=1))
            obounce = octx.enter_context(tc.tile_pool(name="ob", bufs=2))

            # ---- constants (scalar DMA queue - tiny) ----------------------
            lam_t, eu_t = {}, {}
            for g in range(2):
                cs = slice(g * 128, (g + 1) * 128)
                lt = cpool.tile([128, TB], f32, tag=f"lam{g}", name=f"lam{g}")
                nc.scalar.dma_start(lt[:], lamb.ap()[cs, :])
                lam_t[g] = lt
                et = cpool.tile([128, 1], f32, tag=f"eu{g}", name=f"eu{g}")
                nc.scalar.dma_start(et[:], eu_in.ap()[cs, :])
                eu_t[g] = et

            # ---- phase-2 Wo stream: 4 chunks prefetch during phase 1 ------
            wo_r = wo4.ap().rearrange("(ne p) (c j) -> ne p c j",
                                      ne=N_NE, c=N_NE)
            wo_sb = {}

            def load_wo(ne):
                wt = wopool.tile([128, N_NE, 128], f16, tag="wo",
                                 name=f"wo{ne}")
                nc.scalar.dma_start(wt[:], wo_r[ne])
                wo_sb[ne] = wt

            for ne in range(8):
                load_wo(ne)

            with ExitStack() as p1:
                psA = p1.enter_context(
                    tc.tile_pool(name="psA", bufs=8, space="PSUM"))
                wpool = p1.enter_context(tc.tile_pool(name="wpool", bufs=1))
                slab_pool = p1.enter_context(
                    tc.tile_pool(name="slab", bufs=20))
                wkp = p1.enter_context(tc.tile_pool(name="wkv", bufs=2))
                scp = p1.enter_context(tc.tile_pool(name="scan", bufs=2))

                # projection weights (fp16), quarter-chunked and issued
                # q0-of-all-six first so the first matmuls start early
                w_sb = {n: wpool.tile([128, N_NE, C_LOC], f16, tag=n, name=n)
                        for n in W_NAMES}
                w_src = {n: w_dram[n].ap().rearrange("(n p) c -> p n c",
                                                     p=128)
                         for n in W_NAMES}
                for q in range(4):
                    for n in W_NAMES:
                        nc.gpsimd.dma_start(
                            w_sb[n][:, q * 4:(q + 1) * 4, :],
                            w_src[n][:, q * 4:(q + 1) * 4, :])

                def issue_slabs(tb):
                    t0 = tb * TB
                    slabs = []
                    for ne in range(N_NE):
                        s = slab_pool.tile([128, TB + 1], f16, tag="slab",
                                           name=f"slab{tb}_{ne}")
                        nc.sync.dma_start(s[:], xT_r[:, ne, t0:t0 + TB + 1])
                        slabs.append(s)
                    return slabs

                slabs_next = issue_slabs(0)

                carry = {}   # (g, 'n'/'d') -> [128,1] sbuf carry tiles
                for g in range(2):
                    for which in "nd":
                        ct = cpool.tile([128, 1], f32, tag=f"c{which}{g}",
                                        name=f"c{which}{g}")
                        carry[(g, which)] = ct

                projs = (("w1k", "w2k"), ("w1v", "w2v"), ("w1r", "w2r"))

                def wkv_evac(tb, pp):
                    """PSUM-evacuating ops for block tb: head of the next
                    block's ACT/DVE queues frees the 6 banks early."""
                    ev = {}
                    for g in range(2):
                        ek = wkp.tile([128, TB], f32, tag="ek",
                                      name=f"ek{tb}_{g}")
                        nc.scalar.activation(ek[:], pp[(0, g)][:], AF.Exp)
                        emr = wkp.tile([128, TB], f32, tag="emr",
                                       name=f"emr{tb}_{g}")
                        nc.scalar.activation(emr[:], pp[(2, g)][:], AF.Exp,
                                             scale=-1.0)
                        ekv = wkp.tile([128, TB], f32, tag="ekv",
                                       name=f"ekv{tb}_{g}")
                        nc.vector.tensor_mul(ekv[:], ek[:], pp[(1, g)][:])
                        ev[g] = (ek, emr, ekv)
                    return ev

                def wkv_rest(tb, ev):
                    """Post-PSUM WKV math for block tb (SBUF-only)."""
                    for g in range(2):
                        ek, emr, ekv = ev[g]
                        bn = scp.tile([128, TB], f32, tag="bn",
                                      name=f"bn{tb}_{g}")
                        bd = scp.tile([128, TB], f32, tag="bd",
                                      name=f"bd{tb}_{g}")
                        init_n = 0.0 if tb == 0 else carry[(g, "n")][:]
                        init_d = 0.0 if tb == 0 else carry[(g, "d")][:]
                        nc.vector.tensor_tensor_scan(
                            bn[:], lam_t[g][:], ekv[:], init_n,
                            AL.mult, AL.add)
                        nc.vector.tensor_tensor_scan(
                            bd[:], lam_t[g][:], ek[:], init_d,
                            AL.mult, AL.add)

                        num = wkp.tile([128, TB], f32, tag="num",
                                       name=f"num{tb}_{g}")
                        den = wkp.tile([128, TB], f32, tag="den",
                                       name=f"den{tb}_{g}")
                        nc.vector.scalar_tensor_tensor(
                            num[:, 1:TB], ekv[:, 1:TB], eu_t[g][:],
                            bn[:, 0:TB - 1], AL.mult, AL.add)
                        nc.vector.scalar_tensor_tensor(
                            den[:, 1:TB], ek[:, 1:TB], eu_t[g][:],
                            bd[:, 0:TB - 1], AL.mult, AL.add)
                        if tb == 0:
                            nc.vector.tensor_scalar_mul(
                                num[:, 0:1], ekv[:, 0:1], eu_t[g][:])
                            nc.vector.tensor_scalar_mul(
                                den[:, 0:1], ek[:, 0:1], eu_t[g][:])
                        else:
                            nc.vector.scalar_tensor_tensor(
                                num[:, 0:1], ekv[:, 0:1], eu_t[g][:],
                                carry[(g, "n")][:], AL.mult, AL.add)
                            nc.vector.scalar_tensor_tensor(
                                den[:, 0:1], ek[:, 0:1], eu_t[g][:],
                                carry[(g, "d")][:], AL.mult, AL.add)
                        if tb < NTB - 1:
                            nc.vector.tensor_copy(carry[(g, "n")][:],
                                                  bn[:, TB - 1:TB])
                            nc.vector.tensor_copy(carry[(g, "d")][:],
                                                  bd[:, TB - 1:TB])

                        # den2 = den * (1 + e^{-r}) on Pool (sigmoid folded
                        # into the denominator); emr's buffer takes the
                        # product, ekv's (dead by now) takes den2, then one
                        # fast reciprocal (DVE-only custom op) into emr
                        nc.gpsimd.tensor_mul(emr[:], emr[:], den[:])
                        nc.gpsimd.tensor_add(ekv[:], emr[:], den[:])
                        nc.vector.reciprocal_approx_fast(emr[:], ekv[:])
                        y_t = wkp.tile([128, TB], f16, tag="y",
                                       name=f"y{tb}_{g}")
                        nc.vector.tensor_mul(y_t[:], num[:], emr[:])

                        r0 = tb * 128
                        nc.sync.dma_start(a2a_in[g].ap()[r0:r0 + 128, :],
                                          y_t[:])

                pp_prev = None
                for tb in range(NTB):
                    slabs = slabs_next
                    if tb < NTB - 1:
                        slabs_next = issue_slabs(tb + 1)

                    ev_prev = (wkv_evac(tb - 1, pp_prev)
                               if pp_prev is not None else None)

                    pp = {}
                    for pi in range(3):
                        for g in range(2):
                            pp[(pi, g)] = psA.tile(
                                [128, TB], f32, tag="proj",
                                name=f"proj{tb}_{pi}_{g}")

                    for ne in range(N_NE):
                        s = slabs[ne]
                        for pi, (n1, n2) in enumerate(projs):
                            for g in range(2):
                                gsl = slice(g * 128, (g + 1) * 128)
                                nc.tensor.matmul(
                                    pp[(pi, g)][:], w_sb[n1][:, ne, gsl],
                                    s[:, 1:TB + 1],
                                    start=(ne == 0), stop=False)
                                nc.tensor.matmul(
                                    pp[(pi, g)][:], w_sb[n2][:, ne, gsl],
                                    s[:, 0:TB],
                                    start=False, stop=(ne == N_NE - 1))

                    if ev_prev is not None:
                        wkv_rest(tb - 1, ev_prev)
                    pp_prev = pp

                wkv_rest(NTB - 1, wkv_evac(NTB - 1, pp_prev))

            # ---------------- AllToAll + output projection ------------------
            # two collectives (one per channel group): the second pipelines
            # behind the first, and phase 2 starts on g0 channels while the
            # g1 collective is still in flight
            for g in range(2):
                nc.gpsimd.collective_compute(
                    "AllToAll", AL.bypass,
                    replica_groups=[list(range(N_CORES))],
                    ins=[a2a_in[g].ap().opt()], outs=[a2a_out[g].ap().opt()],
                )

            with ExitStack() as p3:
                psB = p3.enter_context(
                    tc.tile_pool(name="psB", bufs=8, space="PSUM"))
                ysh_g = {}
                for g in range(2):
                    yt = yshpool.tile([128, 8, T_SH], f16, tag=f"ysh{g}",
                                      name=f"ysh{g}")
                    ao_r = a2a_out[g].ap().rearrange("(c p) t -> p c t",
                                                     p=128)
                    # evens on sync (free after the y writes); odds on
                    # gpsimd, which is anyway serialized behind the CC1
                    # trigger -- keeps each queue's DMA counter clean so
                    # even-matmuls never wait on odd loads
                    eng = nc.sync if g == 0 else nc.gpsimd
                    for csrc in range(8):
                        eng.dma_start(yt[:, csrc, :], ao_r[:, csrc, :])
                    ysh_g[g] = yt

                def ysh_ap(c):
                    return ysh_g[c % 2][:, c // 2, :]

                evens = list(range(0, N_NE, 2))
                odds = list(range(1, N_NE, 2))
                psb = {}
                for ne in range(8):
                    psb[ne] = psB.tile([128, T_SH], f32, tag="fin",
                                       name=f"fin{ne}")
                    for c in evens:
                        nc.tensor.matmul(
                            psb[ne][:], wo_sb[ne][:, c, :], ysh_ap(c),
                            start=(c == 0), stop=False)

                def finish(ne, corder):
                    ps = psb[ne]
                    for c in corder:
                        nc.tensor.matmul(
                            ps[:], wo_sb[ne][:, c, :], ysh_ap(c),
                            start=False, stop=(c == corder[-1]))
                    b = obounce.tile([128, T_SH], f32, tag="evict",
                                     name=f"ob{ne}")
                    nc.scalar.copy(b[:], ps[:])
                    nc.sync.dma_start(
                        outT.ap()[ne * 128:(ne + 1) * 128, :], b[:])

                for ne in range(8):
                    finish(ne, odds)
                    load_wo(ne + 8)
                for ne in range(8, N_NE):
                    psb[ne] = psB.tile([128, T_SH], f32, tag="fin",
                                       name=f"fin{ne}")
                    for c in evens:
                        nc.tensor.matmul(
                            psb[ne][:], wo_sb[ne][:, c, :], ysh_ap(c),
                            start=(c == 0), stop=False)
                    finish(ne, odds)

    nc.compile()
    return nc


_prog_cache = {}


def _get_program():
    if "nc" not in _prog_cache:
        _prog_cache["nc"] = build_program()
    return _prog_cache["nc"]


def prepare_in_maps(x, time_first, time_decay, time_mix_k, time_mix_v,
                    time_mix_r, Wk, Wv, Wr, Wo):
    x = np.asarray(x, np.float32)
    tmk = np.asarray(time_mix_k, np.float32).reshape(-1)
    tmv = np.asarray(time_mix_v, np.float32).reshape(-1)
    tmr = np.asarray(time_mix_r, np.float32).reshape(-1)
    td = np.asarray(time_decay, np.float64).reshape(-1)
    tf_ = np.asarray(time_first, np.float64).reshape(-1)

    xT = np.zeros((NE, T + 1), np.float16)            # col 0 = shift-in zero
    xT[:, 1:] = x.T.astype(np.float16)
    Wk = np.asarray(Wk, np.float32)
    Wv = np.asarray(Wv, np.float32)
    Wr = np.asarray(Wr, np.float32)
    folded = {
        "w1k": tmk[:, None] * Wk, "w2k": (1.0 - tmk)[:, None] * Wk,
        "w1v": tmv[:, None] * Wv, "w2v": (1.0 - tmv)[:, None] * Wv,
        "w1r": tmr[:, None] * Wr, "w2r": (1.0 - tmr)[:, None] * Wr,
    }
    # wo4[ne*128+p, nc*128+j] = Wo[nc*128+p, ne*128+j]
    wo4 = np.ascontiguousarray(
        np.asarray(Wo, np.float32).astype(np.float16)
        .reshape(N_NE, 128, N_NE, 128).transpose(2, 1, 0, 3)
        .reshape(NE, C))
    # per-step decay multiplier lam_c = exp(-exp(-exp(time_decay_c)))
    lam = np.exp(-np.exp(-np.exp(td))).astype(np.float32)
    eu = np.exp(tf_).astype(np.float32)

    in_maps = []
    for i in range(N_CORES):
        sl = slice(i * C_LOC, (i + 1) * C_LOC)
        im = {
            "xT": xT,
            "wo4": wo4,
            "lamb": np.ascontiguousarray(
                np.broadcast_to(lam[sl, None], (C_LOC, TB))),
            "eu": np.ascontiguousarray(eu[sl, None]),
        }
        for n, W in folded.items():
            im[n] = np.ascontiguousarray(W[:, sl].astype(np.float16))
        in_maps.append(im)
    return in_maps


def run(in_maps, trace=False, **kwargs):
    nc = _get_program()
    res = bass_utils.run_bass_kernel_spmd(
        nc, in_maps, core_ids=list(range(N_CORES)), trace=trace, **kwargs)
    full = np.concatenate(
        [np.asarray(res.results[i]["out"], np.float32).T
         for i in range(N_CORES)], axis=0)
    return full, res


def kernel(**inputs):
    in_maps = prepare_in_maps(**inputs)
    full, _ = run(in_maps, trace=False)
    return full
